# revision 1
# baseline (speedup 1.0000x reference)
"""Per-pixel adaptive 5x5 conv (KPN apply) on 8 Trainium2 NeuronCores.

out[b,c,h,w] = sum_{i,j} core[b,0,i*5+j,c,h,w] * frames[b,0,c,h+i-2,w+j-2]
(zero-padded borders), output [4,3,512,512] f32.

Sharding: pure data parallel, core k -> (b = k//2, H-half = k%2).
Each NeuronCore gets a zero-padded frame shard [3, 260, 516] (halo rows +
W padding done on host), a core shard [25, 3, 256, 512], and produces
[3, 256, 512].

Raw-bass implementation (the walrus build in this env only allows one
semaphore wait per compute/DMA instruction, so Tile's auto-sync can't be
used): explicit double-buffered pipeline, standalone waits (one condition
per instruction), all loads/stores on the SP HWDGE FIFO so ordering among
DMAs is implicit.

Per 128-row block: one DMA brings all 25 tap planes [128, 25, 512], one
DMA brings a 5-row overlapping window of the padded frame [128, 5, 516];
DVE does 25 products (tap row-shift i = window slice, col-shift j =
free-dim element offset) and a serial accumulate chain.
"""

import numpy as np

import concourse.bass as bass
import concourse.mybir as mybir
from concourse.ap import AP
from concourse.bass_utils import run_bass_kernel_spmd

B, N, C, H, W = 4, 1, 3, 512, 512
K = 5
PAD = K // 2
NCORES = 8
HH = H // (NCORES // B)  # 256 rows per core
P = 128
NBLK_TOT = C * (HH // P)  # 6 blocks of 128 rows per core
WPAD = W + 2 * PAD        # 516

_CACHE = {}


def _build():
    nc = bass.Bass()
    f32 = mybir.dt.float32

    fr = nc.declare_dram_parameter("fr", [C, HH + 2 * PAD, WPAD], f32, isOutput=False)
    co = nc.declare_dram_parameter("co", [K * K, C, HH, W], f32, isOutput=False)
    out = nc.declare_dram_parameter("out", [C, HH, W], f32, isOutput=True)

    def co_view(n):
        c, blk = n // (HH // P), n % (HH // P)
        return co[:, c, blk * P:blk * P + P, :].transpose([1, 0, 2])

    def fr_win(n):
        c, blk = n // (HH // P), n % (HH // P)
        fb = fr[c, blk * P:blk * P + P, :]
        return AP(fb.tensor, fb.offset, [(WPAD, P), (WPAD, K), (1, WPAD)])

    def out_view(n):
        c, blk = n // (HH // P), n % (HH // P)
        return out[c, blk * P:blk * P + P, :]

    with (
        nc.sbuf_tensor("ct0", [P, K * K, W], f32) as ct0,
        nc.sbuf_tensor("ct1", [P, K * K, W], f32) as ct1,
        nc.sbuf_tensor("ft0", [P, K, WPAD], f32) as ft0,
        nc.sbuf_tensor("ft1", [P, K, WPAD], f32) as ft1,
        nc.sbuf_tensor("ac0", [P, W], f32) as ac0,
        nc.sbuf_tensor("ac1", [P, W], f32) as ac1,
        nc.sbuf_tensor("tmp", [P, W], f32) as tmp,
        nc.semaphore("dsem") as dsem,   # load completions (+16 per DMA)
        nc.semaphore("osem") as osem,   # store completions (+16 per DMA)
        nc.semaphore("vsem") as vsem,   # DVE per-block completion (+1)
        nc.Block() as block,
    ):
        cts, fts, acs = [ct0, ct1], [ft0, ft1], [ac0, ac1]

        @block.sync
        def _(sync: bass.BassEngine):
            for n in range(NBLK_TOT):
                if n >= 2:
                    # DVE done with block n-2 => its ct/ft buffers reusable,
                    # and acc[n-2] ready to store.
                    sync.wait_ge(vsem, n - 1)
                    sync.dma_start(
                        out=out_view(n - 2), in_=acs[n % 2][:]
                    ).then_inc(osem, 16)
                sync.dma_start(out=cts[n % 2][:], in_=co_view(n)).then_inc(dsem, 16)
                sync.dma_start(out=fts[n % 2][:], in_=fr_win(n)).then_inc(dsem, 16)
            sync.wait_ge(vsem, NBLK_TOT - 1)
            sync.dma_start(
                out=out_view(NBLK_TOT - 2), in_=acs[NBLK_TOT % 2][:]
            ).then_inc(osem, 16)
            sync.wait_ge(vsem, NBLK_TOT)
            sync.dma_start(
                out=out_view(NBLK_TOT - 1), in_=acs[(NBLK_TOT + 1) % 2][:]
            ).then_inc(osem, 16)
            sync.wait_ge(osem, 16 * NBLK_TOT)

        @block.vector
        def _(vector: bass.BassEngine):
            for n in range(NBLK_TOT):
                ct, ft, acc = cts[n % 2], fts[n % 2], acs[n % 2]
                vector.wait_ge(dsem, 32 * (n + 1))
                if n >= 2:
                    # store of block n-2 (same acc buffer) must be done
                    vector.wait_ge(osem, 16 * (n - 1))
                for t in range(K * K):
                    i, j = t // K, t % K
                    csl = ct[:, t, :]
                    fsl = ft[:, i, j:j + W]
                    if t == 0:
                        vector.tensor_mul(out=acc[:], in0=csl, in1=fsl)
                    else:
                        vector.tensor_mul(out=tmp[:], in0=csl, in1=fsl)
                        ins = vector.tensor_add(out=acc[:], in0=acc[:], in1=tmp[:])
                        if t == K * K - 1:
                            ins.then_inc(vsem, 1)
    return nc


def get_nc():
    if "nc" not in _CACHE:
        _CACHE["nc"] = _build()
    return _CACHE["nc"]


def shard_inputs(frames, core):
    frames = np.asarray(frames, dtype=np.float32)
    core = np.asarray(core, dtype=np.float32)
    in_maps = []
    for k in range(NCORES):
        b, half = k // 2, k % 2
        h0 = half * HH
        frp = np.zeros((C, HH + 2 * PAD, WPAD), np.float32)
        lo, hi = h0 - PAD, h0 + HH + PAD
        clo, chi = max(lo, 0), min(hi, H)
        frp[:, clo - lo:clo - lo + chi - clo, PAD:PAD + W] = frames[b, 0, :, clo:chi, :]
        in_maps.append({
            "fr": frp,
            "co": np.ascontiguousarray(core[b, 0, :, :, h0:h0 + HH, :]),
        })
    return in_maps


def run(in_maps, **kwargs):
    return run_bass_kernel_spmd(get_nc(), in_maps, list(range(NCORES)), **kwargs)


def kernel(frames, core):
    in_maps = shard_inputs(frames, core)
    res = run(in_maps).results
    outp = np.empty((B, C, H, W), np.float32)
    for k in range(NCORES):
        b, half = k // 2, k % 2
        outp[b, :, half * HH:(half + 1) * HH, :] = res[k]["out"]
    return outp



# revision 2
# speedup vs baseline: 2.8091x; 2.8091x over previous
"""Per-pixel adaptive 5x5 conv (KPN apply) on 8 Trainium2 NeuronCores.

out[b,c,h,w] = sum_{i,j} core[b,0,i*5+j,c,h,w] * frames[b,0,c,h+i-2,w+j-2]
(zero-padded borders), output [4,3,512,512] f32.

Sharding: pure data parallel, core k -> (b = k//2, H-half = k%2).

v2: all device inputs are bfloat16 (truncated f32 high halves, taken as
zero-copy numpy views on the host so the only host-side gather happens
once, inside the runner). Halves every byte moved: host memcpy, host->
device transfer, and HBM traffic on-core. Kernel math: products and the
accumulate chain in bf16 on DVE (2x packed mode), final add emits f32.
Measured rel err of this scheme vs the f32 reference: ~9e-3 (gate 2e-2).

Raw-bass implementation (the walrus build in this env only allows one
semaphore wait per compute/DMA instruction, so Tile's auto-sync can't be
used): explicit double-buffered pipeline, standalone waits, all loads/
stores on the SP HWDGE FIFO so ordering among DMAs is implicit.

Per 128-row block: one DMA brings all 25 tap planes [128, 25, 512] bf16,
two DMAs bring 5-row overlapping windows of the padded frame (ftA, and
ftB shifted one column so odd-j tap slices stay 4-byte aligned for the
DVE 2x packed mode); DVE does 25 products + serial accumulate in bf16,
last add writes the f32 output tile.
"""

import numpy as np
import ml_dtypes

import concourse.bass as bass
import concourse.mybir as mybir
from concourse.ap import AP
from concourse.bass_utils import run_bass_kernel_spmd

B, N, C, H, W = 4, 1, 3, 512, 512
K = 5
PAD = K // 2
NCORES = 8
HH = H // (NCORES // B)  # 256 rows per core
P = 128
NBLK_TOT = C * (HH // P)  # 6 blocks of 128 rows per core
WPAD = W + 2 * PAD        # 516
BF16 = ml_dtypes.bfloat16

_CACHE = {}


def _build():
    nc = bass.Bass()
    f32 = mybir.dt.float32
    bf16 = mybir.dt.bfloat16

    fr = nc.declare_dram_parameter("fr", [C, HH + 2 * PAD, WPAD], bf16, isOutput=False)
    co = nc.declare_dram_parameter("co", [K * K, C, HH, W], bf16, isOutput=False)
    out = nc.declare_dram_parameter("out", [C, HH, W], f32, isOutput=True)

    def co_view(n):
        c, blk = n // (HH // P), n % (HH // P)
        return co[:, c, blk * P:blk * P + P, :].transpose([1, 0, 2])

    def fr_win(n, shift):
        # [P, K, 516-or-515] window of the padded frame, rows overlapping;
        # shift=1 reads one column later (for odd-j taps).
        c, blk = n // (HH // P), n % (HH // P)
        fb = fr[c, blk * P:blk * P + P, :]
        return AP(fb.tensor, fb.offset + shift,
                  [(WPAD, P), (WPAD, K), (1, WPAD - shift)])

    def out_view(n):
        c, blk = n // (HH // P), n % (HH // P)
        return out[c, blk * P:blk * P + P, :]

    with (
        nc.sbuf_tensor("ct0", [P, K * K, W], bf16) as ct0,
        nc.sbuf_tensor("ct1", [P, K * K, W], bf16) as ct1,
        nc.sbuf_tensor("fa0", [P, K, WPAD], bf16) as fa0,
        nc.sbuf_tensor("fa1", [P, K, WPAD], bf16) as fa1,
        nc.sbuf_tensor("fb0", [P, K, WPAD], bf16) as fb0,
        nc.sbuf_tensor("fb1", [P, K, WPAD], bf16) as fb1,
        nc.sbuf_tensor("ac0", [P, W], bf16) as ac0,
        nc.sbuf_tensor("ac1", [P, W], bf16) as ac1,
        nc.sbuf_tensor("oa0", [P, W], f32) as oa0,
        nc.sbuf_tensor("oa1", [P, W], f32) as oa1,
        nc.sbuf_tensor("tmp", [P, W], bf16) as tmp,
        nc.semaphore("dsem") as dsem,   # load completions (+16 per DMA)
        nc.semaphore("osem") as osem,   # store completions (+16 per DMA)
        nc.semaphore("vsem") as vsem,   # DVE per-block completion (+1)
        nc.Block() as block,
    ):
        cts, fas, fbs = [ct0, ct1], [fa0, fa1], [fb0, fb1]
        acs, oas = [ac0, ac1], [oa0, oa1]

        @block.sync
        def _(sync: bass.BassEngine):
            for n in range(NBLK_TOT):
                if n >= 2:
                    # DVE done with block n-2 => its buffers reusable and
                    # its f32 acc ready to store.
                    sync.wait_ge(vsem, n - 1)
                    sync.dma_start(
                        out=out_view(n - 2), in_=oas[n % 2][:]
                    ).then_inc(osem, 16)
                sync.dma_start(out=cts[n % 2][:], in_=co_view(n)).then_inc(dsem, 16)
                sync.dma_start(out=fas[n % 2][:], in_=fr_win(n, 0)).then_inc(dsem, 16)
                sync.dma_start(
                    out=fbs[n % 2][:, :, 0:WPAD - 1], in_=fr_win(n, 1)
                ).then_inc(dsem, 16)
            sync.wait_ge(vsem, NBLK_TOT - 1)
            sync.dma_start(
                out=out_view(NBLK_TOT - 2), in_=oas[NBLK_TOT % 2][:]
            ).then_inc(osem, 16)
            sync.wait_ge(vsem, NBLK_TOT)
            sync.dma_start(
                out=out_view(NBLK_TOT - 1), in_=oas[(NBLK_TOT + 1) % 2][:]
            ).then_inc(osem, 16)
            sync.wait_ge(osem, 16 * NBLK_TOT)

        @block.vector
        def _(vector: bass.BassEngine):
            for n in range(NBLK_TOT):
                ct, fta, ftb = cts[n % 2], fas[n % 2], fbs[n % 2]
                acc, oac = acs[n % 2], oas[n % 2]
                vector.wait_ge(dsem, 48 * (n + 1))
                if n >= 2:
                    # store of block n-2 (same f32 acc buffer) must be done
                    vector.wait_ge(osem, 16 * (n - 1))
                for t in range(K * K):
                    i, j = t // K, t % K
                    csl = ct[:, t, :]
                    # odd j reads the shifted copy so the slice stays
                    # 4B-aligned (DVE 2x packed mode requirement)
                    fsl = fta[:, i, j:j + W] if j % 2 == 0 \
                        else ftb[:, i, j - 1:j - 1 + W]
                    if t == 0:
                        vector.tensor_mul(out=acc[:], in0=csl, in1=fsl)
                    elif t < K * K - 1:
                        vector.tensor_mul(out=tmp[:], in0=csl, in1=fsl)
                        vector.tensor_add(out=acc[:], in0=acc[:], in1=tmp[:])
                    else:
                        vector.tensor_mul(out=tmp[:], in0=csl, in1=fsl)
                        vector.tensor_add(
                            out=oac[:], in0=acc[:], in1=tmp[:]
                        ).then_inc(vsem, 1)
    return nc


def get_nc():
    if "nc" not in _CACHE:
        _CACHE["nc"] = _build()
    return _CACHE["nc"]


def _as_bf16_trunc(a):
    # Zero-copy bf16 view: the high 16 bits of each f32 (little-endian).
    # Truncation (not round-to-nearest); max rel err 2^-8 per element.
    return a.view(np.uint16)[..., 1::2].view(BF16)


def shard_inputs(frames, core):
    frames = np.asarray(frames)
    core = np.asarray(core)
    assert frames.dtype == np.float32 and core.dtype == np.float32
    fr_bf = _as_bf16_trunc(frames)  # [B,1,C,H,W] bf16 view
    co_bf = _as_bf16_trunc(core)    # [B,1,25,C,H,W] bf16 view
    # One small padded copy per batch (the halo rows / W padding); all
    # per-core entries below are views, so the only large gather happens
    # once, inside the runner (concat / tobytes).
    fp = np.zeros((B, C, H + 2 * PAD, WPAD), BF16)
    fp[:, :, PAD:PAD + H, PAD:PAD + W] = fr_bf[:, 0]
    in_maps = []
    for k in range(NCORES):
        b, half = k // 2, k % 2
        h0 = half * HH
        in_maps.append({
            "fr": fp[b, :, h0:h0 + HH + 2 * PAD, :],
            "co": co_bf[b, 0, :, :, h0:h0 + HH, :],
        })
    return in_maps


def run(in_maps, **kwargs):
    return run_bass_kernel_spmd(get_nc(), in_maps, list(range(NCORES)), **kwargs)


def kernel(frames, core):
    in_maps = shard_inputs(frames, core)
    res = run(in_maps).results
    outp = np.empty((B, C, H, W), np.float32)
    for k in range(NCORES):
        b, half = k // 2, k % 2
        outp[b, :, half * HH:(half + 1) * HH, :] = res[k]["out"]
    return outp
